# revision 31
# baseline (speedup 1.0000x reference)
"""GAT (3-layer) Trainium2 Bass kernel, 8-way node-sharded. v2.

Self-contained: host preprocessing (graph partitioning, relabeling, edge
stream construction) + Bass/Tile kernel + gather/unshard.

Strategy (v2):
  - Relabel nodes so core c owns new ids [c*NLOC, (c+1)*NLOC); blocks of 128
    dst nodes; per-block uniform chunk quotas (TE even-src + TO odd-src
    chunks of 128 edges each) with sentinel padding.
  - Single pair-packed gather table per layer (row = even|odd node payload,
    stride 640 bf16 cols L0/L1, 256 cols L2). One AllGather per layer, split
    in two halves so the first half overlaps the tail of the previous layer's
    edge phase. Table rows hold [4x(64 feats + ones) | s_src f32] per node.
  - Per-edge s_dst is computed ON-CHIP: a transposed one-hot (mT, built from
    a partition-broadcast dslotT stream + per-partition iota is_equal) times
    the block's s_dst vector (kept in a persistent SBUF tile) on the PE.
    This removes the per-edge s_dst DMA gather (desc-rate-bound) entirely.
  - Edge phase: dma_gather of per-edge rows (parity = column offset in the
    pair row), exp(lrelu(s_src+s_dst)) per edge, features scaled by per-head
    ee, one-hot matmul on PE accumulates per-dst sums + softmax denominators
    in PSUM.
  - Finalize per block: normalize, bias+BN affine, ELU, dense matmul for the
    next layer via PE transposes, next-layer attention scalars via W@A.
"""
import numpy as np
import ml_dtypes
from contextlib import ExitStack

NO_EE = [False]

import concourse.bacc as bacc
import concourse.bass as bass
import concourse.mybir as mybir
import concourse.tile as tile
from concourse.bass_utils import run_bass_kernel_spmd

P = 128
NCORES = 8
EPS_BN = 1e-5
NEG = -1e38
F32 = mybir.dt.float32
BF16 = mybir.dt.bfloat16
I16 = mybir.dt.int16
BF = ml_dtypes.bfloat16

# Full-problem constants (matches reference.py / spec.json)
N_FULL, E_FULL, F_IN, HID, HEADS, OUTD = 50000, 800000, 128, 64, 4, 2


# ----------------------------------------------------------------------------
# Host preprocessing
# ----------------------------------------------------------------------------

def preprocess(edge_index, n_nodes, nblk, te, to):
    """Graph partitioning + edge stream construction.

    Returns (new_id [n_nodes], plans per core). All integer index work only.
    """
    T = te + to
    nloc = nblk * P
    cap_e, cap_o = te * P, to * P
    src = np.concatenate([edge_index[0].astype(np.int64), np.arange(n_nodes)])
    dst = np.concatenate([edge_index[1].astype(np.int64), np.arange(n_nodes)])
    deg = np.bincount(dst, minlength=n_nodes)

    # nodes -> cores, balancing in-edge load
    order = np.argsort(-deg, kind="stable")
    cap_nodes = n_nodes // NCORES
    core_of = np.empty(n_nodes, np.int64)
    loads = np.zeros(NCORES, np.int64)
    counts = np.zeros(NCORES, np.int64)
    for n in order:
        avail = np.flatnonzero(counts < cap_nodes)
        c = avail[np.argmin(loads[avail])]
        core_of[n] = c
        loads[c] += deg[n]
        counts[c] += 1

    # per core: nodes -> blocks, balancing block load
    new_id = np.full(n_nodes, -1, np.int64)
    for c in range(NCORES):
        nodes = np.flatnonzero(core_of == c)
        nodes = nodes[np.argsort(-deg[nodes], kind="stable")]
        bload = np.zeros(nblk, np.int64)
        bcount = np.zeros(nblk, np.int64)
        slot_ctr = np.zeros(nblk, np.int64)
        for n in nodes:
            avail = np.flatnonzero(bcount < P)
            b = avail[np.argmin(bload[avail])]
            bload[b] += deg[n]
            bcount[b] += 1
            new_id[n] = c * nloc + b * P + slot_ctr[b]
            slot_ctr[b] += 1
        if bload.max() > T * P:
            raise RuntimeError(f"block overflow: {bload.max()} > {T*P}")

    nsrc = new_id[src]
    ndst = new_id[dst]
    ngb = NCORES * nblk

    # parity repair: per global block, even-src count <= cap_e, odd <= cap_o
    rng = np.random.default_rng(1)
    for _ in range(20000):
        gblk = ndst // P
        ec = np.bincount(gblk[(nsrc & 1) == 0], minlength=ngb)
        tc = np.bincount(gblk, minlength=ngb)
        bad = np.flatnonzero((ec > cap_e) | ((tc - ec) > cap_o))
        if not len(bad):
            break
        b = bad[0]
        par = 0 if ec[b] > cap_e else 1
        eidx = np.flatnonzero(gblk == b)
        cand = eidx[(nsrc[eidx] & 1) == par]
        sn = nsrc[cand[rng.integers(len(cand))]]
        sb = (sn // P) * P
        partners = np.arange(sb + (1 - par), sb + P, 2)
        pn = partners[rng.integers(len(partners))]
        for arr in (nsrc, ndst):
            ms, mp = arr == sn, arr == pn
            arr[ms] = pn
            arr[mp] = sn
        os_ = np.flatnonzero(new_id == sn)[0]
        op_ = np.flatnonzero(new_id == pn)[0]
        new_id[os_], new_id[op_] = pn, sn
    else:
        raise RuntimeError("parity repair failed")

    # per-core streams in tile order:
    # group-major; within a group of G blocks: [all even chunks | all odd chunks]
    plans = []
    for c in range(NCORES):
        sel = (ndst >= c * nloc) & (ndst < (c + 1) * nloc)
        es, ed = nsrc[sel], ndst[sel] - c * nloc
        b = ed // P
        par = (es & 1).astype(np.int64)
        o = np.lexsort((es, par, b))
        es, ed, b, par = es[o], ed[o], b[o], par[o]
        # per-block, per-parity slot ranges (block-major natural order first)
        idx_e = np.full((nblk, cap_e), -1, np.int64)
        ds_e = np.zeros((nblk, cap_e), np.int64)
        idx_o = np.full((nblk, cap_o), -1, np.int64)
        ds_o = np.zeros((nblk, cap_o), np.int64)
        for blk in range(nblk):
            m = b == blk
            for pp, idx_a, ds_a, cap in ((0, idx_e, ds_e, cap_e), (1, idx_o, ds_o, cap_o)):
                mm = m & (par == pp)
                k = int(mm.sum())
                assert k <= cap
                idx_a[blk, :k] = es[mm]
                ds_a[blk, :k] = ed[mm] % P
        plans.append(dict(idx_e=idx_e, ds_e=ds_e, idx_o=idx_o, ds_o=ds_o))
    return new_id, plans


def _wrap_idx(a):
    """[k] int -> wrapped [128, k/16] int16 (16-partition wrap, replicated x8)."""
    w = a.reshape(-1, 16).T.astype(np.int16)
    return np.ascontiguousarray(np.tile(w, (8, 1)))


# ----------------------------------------------------------------------------
# Bass kernel builder (v2)
# ----------------------------------------------------------------------------

def build_kernel(cfg):
    """cfg: dict(nblk, te, to, g, n_nodes[, h1b, repeat, noop, skip_gather,
    skip_ag]). Returns nc."""
    nblk, te, to, G = cfg["nblk"], cfg["te"], cfg["to"], cfg["g"]
    h1b = cfg["h1b"]                 # blocks in AG half 1 (split at group bdry)
    skip_ag = cfg.get("skip_ag", False)
    skip_gather = cfg.get("skip_gather", False)
    repeat = cfg.get("repeat", 1)
    noop = cfg.get("noop", False)
    pool_scale = cfg.get("pool_scale", False)
    no_scale = cfg.get("no_scale", False)
    no_m = cfg.get("no_m", False)
    T = te + to
    nloc = nblk * P
    npair_c = nloc // 2              # pairs per core (incl dummies)
    npair = NCORES * npair_c         # pair rows in the table (excl sentinel)
    h1 = h1b * (P // 2)              # pair rows per core in AG half 1
    h2 = npair_c - h1
    groups = [(i, min(i + G, nblk)) for i in range(0, nblk, G)]

    NODEW = [268, 268, 68]           # bf16 cols of one node's payload
    STRIDE = [640, 640, 256]         # pair-row stride (bf16 cols)
    ELEM = [384, 384, 128]           # gather elem (bf16 cols)
    SOFF = [260, 260, 66]            # s_src f32 at bf16 col (4B aligned)
    NCH = [HEADS * (HID + 1), HEADS * (HID + 1), HID + 1]  # agg matmul width
    NH = [HEADS, HEADS, 1]

    nc = bacc.Bacc("TRN2", num_devices=NCORES, num_swdge_queues=4)
    dt = nc.dram_tensor

    # ---- inputs
    x_in = dt("x_slice", [nloc, F_IN], F32, kind="ExternalInput")
    W_in = [dt(f"W{l}", [F_IN if l == 0 else HEADS * HID, (HEADS if l < 2 else 1) * HID], F32, kind="ExternalInput") for l in range(3)]
    Wc_in = dt("Wc", [HID, OUTD], F32, kind="ExternalInput")
    A_in = [dt(f"A{l}", [(HEADS if l < 2 else 1) * HID, 2 * (HEADS if l < 2 else 1)], F32, kind="ExternalInput") for l in range(3)]
    bn_in = [dt(f"bn{l}", [5, (HEADS if l < 2 else 1) * HID], F32, kind="ExternalInput") for l in range(3)]  # rows: b,g,bt,m,v
    bc_in = dt("bc", [1, OUTD], F32, kind="ExternalInput")
    ident_in = dt("ident", [P, P], F32, kind="ExternalInput")
    iota_in = dt("iota", [P, P], BF16, kind="ExternalInput")

    n_e = te * P
    n_o = to * P
    idx_e_in = dt("idx_e", [P, nblk * n_e // 16], I16, kind="ExternalInput")
    idx_o_in = dt("idx_o", [P, nblk * n_o // 16], I16, kind="ExternalInput")
    dslot_in = dt("dslot", [P, nblk * T], BF16, kind="ExternalInput")
    mT_in = dt("mT", [P, nblk * T * P], BF16, kind="ExternalInput")

    y_out = dt("y", [nloc, OUTD], F32, kind="ExternalOutput")

    # ---- internal DRAM: flat pair tables + staging
    tabf = [dt(f"tab{l}", [1, (npair + 3) * STRIDE[l]], BF16, kind="Internal",
               addr_space="Shared") for l in range(3)]
    stg = [dt(f"stg{l}", [npair_c, STRIDE[l]], BF16, kind="Internal") for l in range(3)]

    rg = [list(range(NCORES))]

    if noop:
        with tile.TileContext(nc) as tc, ExitStack() as ctx:
            p0 = ctx.enter_context(tc.tile_pool(name="noop", bufs=1))
            t0 = p0.tile([P, OUTD], F32)
            nc.vector.memset(t0[:], 0.0)
            nc.sync.dma_start(y_out[0:P, :], t0[:])
        nc.compile()
        return nc

    def tab_rows(l, coloff, elemw):
        """View of tabf[l] as pair rows [npair+1, elemw] starting at coloff."""
        s = STRIDE[l]
        return tabf[l][0:1, coloff:coloff + (npair + 1) * s].rearrange(
            "o (n w) -> (o n) w", w=s)[:, 0:elemw]

    with tile.TileContext(nc) as tc, ExitStack() as ctx:
        cst = ctx.enter_context(tc.tile_pool(name="cst", bufs=1))
        wrk = ctx.enter_context(tc.tile_pool(name="wrk", bufs=2))
        pk = ctx.enter_context(tc.tile_pool(name="pk", bufs=2))
        ps = ctx.enter_context(tc.tile_pool(name="ps", bufs=2, space="PSUM"))
        ps1 = ctx.enter_context(tc.tile_pool(name="ps1", bufs=1, space="PSUM"))
        psd = ctx.enter_context(tc.tile_pool(name="psd", bufs=2, space="PSUM"))
        setup_ctx = ExitStack()
        stp = setup_ctx.enter_context(tc.tile_pool(name="stp", bufs=1))

        # ---- constants to SBUF
        ident = cst.tile([P, P], F32)
        nc.sync.dma_start(ident[:], ident_in[:])
        ones1 = cst.tile([1, P], BF16)
        nc.vector.memset(ones1[:], 1.0)

        def bcast_row(dst_sb, row_ap, w):
            # dst_sb[p, :w] = row_ap[0, :w] for all partitions, via K=1 matmul
            bp = ps1.tile([P, 512], F32, tag="tph")
            rb = stp.tile([1, 512], BF16, tag="rbf")
            nc.vector.tensor_copy(rb[:, :w], row_ap)
            nc.tensor.matmul(bp[:, :w], ones1[:], rb[:, :w], start=True, stop=True)
            nc.vector.tensor_copy(dst_sb, bp[:, :w])
        iota = cst.tile([P, P], BF16)
        nc.sync.dma_start(iota[:], iota_in[:])
        dslot_sb = cst.tile([P, nblk * T], BF16)
        nc.sync.dma_start(dslot_sb[:], dslot_in[:])
        bc_rep = cst.tile([P, OUTD], F32)
        bcl = stp.tile([1, OUTD], F32, tag="bcl")
        nc.sync.dma_start(bcl[:], bc_in[:])
        bcast_row(bc_rep[:], bcl[0:1, :], OUTD)

        # persistent per-block s_dst (bf16), overwritten each layer
        sdst_all = cst.tile([P, nblk, HEADS], BF16)

        # per-layer weight tiles (bf16) + WA + BN affine
        Wt = []     # list of [K-chunk bf16 tiles]  (rhs for dense)
        WAt = []    # [K-chunk [128, 2*nh] bf16]
        sc_rep = []
        sh_rep = []
        for l in range(3):
            fin = F_IN if l == 0 else HEADS * HID
            fout = (HEADS if l < 2 else 1) * HID
            nkin = fin // P
            # load W f32, cast to bf16 chunk tiles
            wf = stp.tile([P, (fin // P) * fout], F32, tag=f"wload{l}")
            nc.sync.dma_start(wf[:].rearrange("p (k f) -> p k f", k=nkin),
                              W_in[l].rearrange("(k p) f -> p k f", p=P))
            wb = cst.tile([P, (fin // P) * fout], BF16, tag=f"wb{l}")
            nc.vector.tensor_copy(wb[:], wf[:])
            Wt.append([wb[:, k * fout:(k + 1) * fout] for k in range(nkin)])

            # WT chunks (f32) for WA = W.T-free computation: WA[fin,2nh]
            nchT = (fout + P - 1) // P
            wT = stp.tile([P, nchT * fin], F32, tag=f"wT{l}")
            for ki in range(nkin):          # W row chunk (fin dim)
                for kj in range(nchT):      # W col chunk (fout dim)
                    cw = min(P, fout - kj * P)
                    tp = ps1.tile([P, 512], F32, tag="tph")
                    nc.tensor.transpose(tp[:cw, :P], wf[:, ki * fout + kj * P: ki * fout + kj * P + cw], ident[:])
                    nc.vector.tensor_copy(wT[:cw, kj * fin + ki * P: kj * fin + (ki + 1) * P], tp[:cw, :P])
            nh = NH[l]
            fo_p = min(P, fout)
            af = stp.tile([P, nchT * 2 * nh], F32, tag=f"aload{l}")
            if fout >= P:
                nc.sync.dma_start(af[:].rearrange("p (k f) -> p k f", k=nchT),
                                  A_in[l].rearrange("(k p) f -> p k f", p=P))
            else:
                nc.sync.dma_start(af[:fo_p, 0:2 * nh], A_in[l][:])
            # WA [fin, 2nh] = sum_kj WT_chunk.T @ A_chunk   (bf16 result)
            wab = cst.tile([P, nkin * 2 * nh], BF16, tag=f"wab{l}")
            for ki in range(nkin):
                wa_ps = ps1.tile([P, 2 * nh], F32, tag="sps")
                for kj in range(nchT):
                    cw = min(P, fout - kj * P)
                    nc.tensor.matmul(wa_ps[:],
                                     wT[:cw, kj * fin + ki * P: kj * fin + (ki + 1) * P],
                                     af[:cw, kj * 2 * nh:(kj + 1) * 2 * nh],
                                     start=(kj == 0), stop=(kj == nchT - 1))
                nc.vector.tensor_copy(wab[:, ki * 2 * nh:(ki + 1) * 2 * nh], wa_ps[:])
            WAt.append([wab[:, k * 2 * nh:(k + 1) * 2 * nh] for k in range(nkin)])

            # BN affine: scale2 = g/sqrt(v+eps); shift2 = (b - m)*scale2 + bt
            bn = stp.tile([1, 5 * fout], F32, tag=f"bnload{l}")
            nc.sync.dma_start(bn[:].rearrange("p (r f) -> p r f", r=5), bn_in[l][None, :, :])
            bnr = [bn[:, i * fout:(i + 1) * fout] for i in range(5)]  # b,g,bt,m,v
            sc1 = stp.tile([1, fout], F32, tag=f"sc1{l}")
            nc.vector.tensor_scalar(out=sc1[:], in0=bnr[4], scalar1=EPS_BN, scalar2=None, op0=mybir.AluOpType.add)
            nc.scalar.activation(sc1[:], sc1[:], mybir.ActivationFunctionType.Sqrt)
            rc = stp.tile([1, fout], F32, tag=f"rc1{l}")
            nc.vector.reciprocal(rc[:], sc1[:])
            nc.vector.tensor_tensor(out=rc[:], in0=rc[:], in1=bnr[1], op=mybir.AluOpType.mult)
            sh1 = stp.tile([1, fout], F32, tag=f"sh1{l}")
            nc.vector.tensor_tensor(out=sh1[:], in0=bnr[0], in1=bnr[3], op=mybir.AluOpType.subtract)
            nc.vector.tensor_tensor(out=sh1[:], in0=sh1[:], in1=rc[:], op=mybir.AluOpType.mult)
            nc.vector.tensor_tensor(out=sh1[:], in0=sh1[:], in1=bnr[2], op=mybir.AluOpType.add)
            screp = cst.tile([P, fout], F32, tag=f"screp{l}")
            bcast_row(screp[:], rc[0:1, :], fout)
            shrep = cst.tile([P, fout], F32, tag=f"shrep{l}")
            bcast_row(shrep[:], sh1[0:1, :], fout)
            sc_rep.append(screp)
            sh_rep.append(shrep)

        wcf = stp.tile([HID, OUTD], F32, tag="wcl")
        nc.sync.dma_start(wcf[:], Wc_in[:])
        Wct = cst.tile([HID, OUTD], BF16)
        nc.vector.tensor_copy(Wct[:], wcf[:])

        # ---- sentinel rows: zero payload, s_src slots NEG (both parities)
        for l in range(3):
            s = STRIDE[l]
            sent = stp.tile([1, s], BF16, tag=f"sent{l}")
            nc.vector.memset(sent[:], 0)
            s32 = sent[:].bitcast(F32)
            nhl = NH[l]
            nc.vector.memset(s32[:, SOFF[l] // 2: SOFF[l] // 2 + nhl], NEG)
            nc.vector.memset(s32[:, (NODEW[l] + SOFF[l]) // 2: (NODEW[l] + SOFF[l]) // 2 + nhl], NEG)
            nc.sync.dma_start(
                tabf[l][0:1, npair * s:(npair + 1) * s], sent[:])
            # zero the spill pad row too (keeps over-reads NaN-free)
            nc.vector.memset(sent[:], 0)
            nc.sync.dma_start(tabf[l][0:1, (npair + 1) * s:(npair + 2) * s], sent[:])

        # ================= per-layer finalize helper ====================
        def finalize_block(l, zsrc_ps, b, b_in_grp, pack, pack_y):
            """zsrc_ps: agg psum [P, NCH[l]] (cols: per-head 64 feats + denom).
            Produces next-layer packed node rows into pack (this group's
            staging tile), or y into pack_y for l==2. Writes sdst_all[:, b]."""
            nh = NH[l]
            fout = nh * HID
            # denominators -> reciprocal
            dn = wrk.tile([P, nh], F32, tag=f"dn{nh}")
            dcols = zsrc_ps[:].rearrange("p (h c) -> p h c", c=HID + 1)[:, :, HID]
            nc.vector.tensor_scalar(out=dn[:], in0=dcols, scalar1=1e-30, scalar2=None, op0=mybir.AluOpType.add)
            rd = wrk.tile([P, nh], F32, tag=f"rd{nh}")
            nc.vector.reciprocal(rd[:], dn[:])
            # normalize * BN-scale fused, then + shift
            h = wrk.tile([P, fout], F32, tag=f"h{fout}")
            for hd in range(nh):
                nc.vector.scalar_tensor_tensor(
                    out=h[:, hd * HID:(hd + 1) * HID],
                    in0=zsrc_ps[:, hd * (HID + 1):hd * (HID + 1) + HID],
                    scalar=rd[:, hd, None],
                    in1=sc_rep[l][:, hd * HID:(hd + 1) * HID],
                    op0=mybir.AluOpType.mult, op1=mybir.AluOpType.mult)
            nc.vector.tensor_tensor(out=h[:], in0=h[:], in1=sh_rep[l][:], op=mybir.AluOpType.add)
            if l < 2:
                u = wrk.tile([P, fout], F32, tag=f"elu_u{fout}")
                nc.vector.tensor_scalar(out=u[:], in0=h[:], scalar1=0.0, scalar2=None, op0=mybir.AluOpType.min)
                nc.scalar.activation(u[:], u[:], mybir.ActivationFunctionType.Exp)
                nc.vector.tensor_scalar(out=h[:], in0=h[:], scalar1=0.0, scalar2=-1.0, op0=mybir.AluOpType.max, op1=mybir.AluOpType.add)
                nc.vector.tensor_tensor(out=h[:], in0=h[:], in1=u[:], op=mybir.AluOpType.add)
            # transpose h -> hT bf16 chunks
            nkin = fout // P if fout >= P else 1
            hT = wrk.tile([P, nkin * P], BF16, tag=f"hT{fout}")
            for k in range(nkin):
                cw = min(P, fout - k * P)
                tp = ps1.tile([P, 512], F32, tag="tph")
                nc.tensor.transpose(tp[:cw, :P], h[:, k * P:k * P + cw], ident[:])
                nc.scalar.copy(hT[:cw, k * P:(k + 1) * P], tp[:cw, 0:P])
            if l == 2:
                # head: y = h2 @ Wc + bc
                yp = ps1.tile([P, OUTD], F32, tag="sps")
                nc.tensor.matmul(yp[:], hT[:HID, 0:P], Wct[:], start=True, stop=True)
                nc.vector.tensor_tensor(out=pack_y[:, b_in_grp, :], in0=yp[:], in1=bc_rep[:], op=mybir.AluOpType.add)
                return
            # dense: hf_next = h @ W_{l+1}; s = h @ WA_{l+1}   (both bf16 hT)
            nl = l + 1
            nhn = NH[nl]
            fon = nhn * HID
            hf = ps.tile([P, fon], F32, tag="hfps")
            sps = ps1.tile([P, 2 * nhn], F32, tag="sps")
            for k in range(nkin):
                nc.tensor.matmul(hf[:], hT[:, k * P:(k + 1) * P], Wt[nl][k], start=(k == 0), stop=(k == nkin - 1))
            for k in range(nkin):
                nc.tensor.matmul(sps[:], hT[:, k * P:(k + 1) * P], WAt[nl][k], start=(k == 0), stop=(k == nkin - 1))
            # pack node rows: [hf_hd | 1.0]*nh + s_src f32
            nc.scalar.copy(
                pack[:, b_in_grp, 0:nhn * (HID + 1)].rearrange("p (h c) -> p h c", h=nhn)[:, :, 0:HID],
                hf[:].rearrange("p (h c) -> p h c", h=nhn))
            pk32 = pack[:].bitcast(F32)
            so = SOFF[nl] // 2
            nc.vector.tensor_copy(pk32[:, b_in_grp, so:so + nhn], sps[:, 0:nhn])
            # next layer's s_dst -> persistent sbuf (bf16)
            nc.vector.tensor_copy(sdst_all[:, b, 0:nhn], sps[:, nhn:2 * nhn])

        def stage_group(l, g0, gn, pack):
            """Write group's pack (node rows) into stg[l] pair rows."""
            nw = NODEW[l]
            for par in (0, 1):
                nc.sync.dma_start(
                    stg[l][g0 * 64:g0 * 64 + gn * 64, par * nw:(par + 1) * nw]
                    .rearrange("(g q) w -> q g w", g=gn),
                    pack[par::2, 0:gn, 0:nw])

        def do_ag(l):
            s = STRIDE[l]
            if skip_ag:
                nc.sync.dma_start(
                    tabf[l][0:1, 0:npair_c * s].rearrange("o (n w) -> (o n) w", w=s),
                    stg[l][:])
                return
            nc.gpsimd.collective_compute(
                "AllGather", mybir.AluOpType.bypass,
                ins=[stg[l][0:h1, :]],
                outs=[tabf[l][0:1, 0:NCORES * h1 * s]], replica_groups=rg)
            nc.gpsimd.collective_compute(
                "AllGather", mybir.AluOpType.bypass,
                ins=[stg[l][h1:npair_c, :]],
                outs=[tabf[l][0:1, NCORES * h1 * s:NCORES * npair_c * s]],
                replica_groups=rg)

        setup_ctx.close()

        for _rep in range(repeat):
            # ================= L0 prep: x -> table0 rows =====================
            for g0, g1 in groups:
                gn = g1 - g0
                pack = pk.tile([P, G, NODEW[0]], BF16, tag="pack0")
                nc.vector.memset(pack[:, :gn, :], 0)
                for b in range(g0, g1):
                    bi = b - g0
                    xt = wrk.tile([P, F_IN], F32, tag="xt")
                    nc.sync.dma_start(xt[:], x_in[b * P:(b + 1) * P, :])
                    xT = wrk.tile([P, F_IN], BF16, tag="xT")
                    tp = ps1.tile([P, 512], F32, tag="tph")
                    nc.tensor.transpose(tp[:, 0:P], xt[:], ident[:])
                    nc.scalar.copy(xT[:], tp[:, 0:P])
                    hf = ps.tile([P, HEADS * HID], F32, tag="hfps")
                    sps = ps1.tile([P, 2 * HEADS], F32, tag="sps")
                    nc.tensor.matmul(hf[:], xT[:], Wt[0][0], start=True, stop=True)
                    nc.tensor.matmul(sps[:], xT[:], WAt[0][0], start=True, stop=True)
                    nc.scalar.copy(
                        pack[:, bi, 0:HEADS * (HID + 1)].rearrange("p (h c) -> p h c", h=HEADS)[:, :, 0:HID],
                        hf[:].rearrange("p (h c) -> p h c", h=HEADS))
                    pk32 = pack[:].bitcast(F32)
                    so = SOFF[0] // 2
                    nc.vector.tensor_copy(pk32[:, bi, so:so + HEADS], sps[:, 0:HEADS])
                    nc.vector.tensor_copy(sdst_all[:, b, :], sps[:, HEADS:2 * HEADS])
                # ones cols
                on = pack[:, 0:gn, 0:HEADS * (HID + 1)].rearrange("p g (h c) -> p g h c", h=HEADS)[:, :, :, HID:HID + 1]
                nc.vector.memset(on, 1.0)
                stage_group(0, g0, gn, pack)

            # ================= layers =====================
            for l in range(3):
                nh = NH[l]
                elem = ELEM[l]
                nch_w = NCH[l]
                soff32 = SOFF[l] // 2
                do_ag(l)

                for gi, (g0, g1) in enumerate(groups):
                    gn = g1 - g0
                    nche = gn * te       # even chunks in group
                    ncho = gn * to
                    ncht = nche + ncho
                    # chunk order in tiles: [even chunks | odd chunks]
                    ixe = wrk.tile([P, nche * 8], I16, tag="ixe")
                    nc.sync.dma_start(ixe[:], idx_e_in[:, g0 * n_e // 16:(g0 * n_e + nche * P) // 16])
                    ixo = wrk.tile([P, ncho * 8], I16, tag="ixo")
                    nc.sync.dma_start(ixo[:], idx_o_in[:, g0 * n_o // 16:(g0 * n_o + ncho * P) // 16])

                    # mT (one-hot, dst on partitions) streamed from host
                    mt = wrk.tile([P, ncht, P], BF16, tag="mt")
                    nc.sync.dma_start(mt[:].rearrange("p c q -> p (c q)"),
                                      mT_in[:, g0 * T * P:(g0 * T + ncht) * P])

                    g = wrk.tile([P, G * T, elem], BF16, tag="g")
                    if skip_gather:
                        nc.vector.memset(g[:, 0:ncht, :], 0)
                    else:
                        nc.gpsimd.dma_gather(g[:, 0:nche, :], tab_rows(l, 0, elem),
                                             ixe[:], nche * P, nche * P, elem,
                                             elem_step=STRIDE[l], single_packet=False,
                                             queue_num=(2 * gi) % 4)
                        nc.gpsimd.dma_gather(g[:, nche:ncht, :], tab_rows(l, NODEW[l], elem),
                                             ixo[:], ncho * P, ncho * P, elem,
                                             elem_step=STRIDE[l], single_packet=False,
                                             queue_num=(2 * gi + 1) % 4)

                    # per-edge s_dst via mT @ sdst_blk on PE
                    no_ee = NO_EE[0]
                    sdp = psd.tile([P, G * T * HEADS], F32, tag="sdp")
                    def blk_of(ci):
                        return g0 + (ci // te if ci < nche else (ci - nche) // to)
                    for ci in range(0 if not no_ee else ncht, ncht):
                        nc.tensor.matmul(sdp[:, ci * nh:(ci + 1) * nh],
                                         mt[:, ci, :], sdst_all[:, blk_of(ci), 0:nh],
                                         start=True, stop=True)
                    sde = wrk.tile([P, G * T, HEADS], F32, tag="sde")
                    if not no_ee:
                        nc.scalar.copy(sde[:, 0:ncht, 0:nh].rearrange("p c h -> p (c h)"),
                                   sdp[:, 0:ncht * nh])

                    # e = lrelu(s_src + s_dst); ee = exp(e)  (bf16)
                    g32 = g[:].bitcast(F32)
                    e_t = wrk.tile([P, G * T, HEADS], F32, tag="e_t")
                    if not no_ee:
                        nc.vector.tensor_tensor(out=e_t[:, 0:ncht, 0:nh], in0=sde[:, 0:ncht, 0:nh],
                                                in1=g32[:, 0:ncht, soff32:soff32 + nh], op=mybir.AluOpType.add)
                        nc.vector.scalar_tensor_tensor(out=e_t[:, 0:ncht, 0:nh], in0=e_t[:, 0:ncht, 0:nh],
                                                       scalar=0.2, in1=e_t[:, 0:ncht, 0:nh],
                                                       op0=mybir.AluOpType.mult, op1=mybir.AluOpType.max)
                    ee = wrk.tile([P, G * T, HEADS], BF16, tag="ee")
                    if no_ee:
                        nc.vector.memset(ee[:, 0:ncht, 0:nh], 1.0)
                    else:
                        nc.scalar.activation(ee[:, 0:ncht, 0:nh], e_t[:, 0:ncht, 0:nh], mybir.ActivationFunctionType.Exp)

                    # scale gathered features by per-head ee (in place),
                    # split across DVE and Pool
                    for hd in range(0 if not no_scale else nh, nh):
                        eng = nc.gpsimd if (pool_scale and nh > 1 and hd >= 2) else nc.vector
                        eng.tensor_tensor(
                            out=g[:, 0:ncht, hd * (HID + 1):(hd + 1) * (HID + 1)],
                            in0=g[:, 0:ncht, hd * (HID + 1):(hd + 1) * (HID + 1)],
                            in1=ee[:, 0:ncht, hd, None].to_broadcast([P, ncht, HID + 1]),
                            op=mybir.AluOpType.mult)

                    # one-hot m (edges on partitions) for aggregation
                    m = wrk.tile([P, G * T, P], BF16, tag="m")
                    if no_m:
                        nc.vector.memset(m[:, 0:ncht, :], 0)
                    else:
                        nc.vector.tensor_tensor(
                            out=m[:, 0:ncht, :],
                            in0=iota[:, None, :].to_broadcast([P, ncht, P]),
                            in1=dslot_sb[:, (g0 * T):(g0 * T) + ncht, None].to_broadcast([P, ncht, P]),
                            op=mybir.AluOpType.is_equal)

                    # aggregate per block
                    if l < 2:
                        pack = pk.tile([P, G, NODEW[l + 1]], BF16, tag=f"packL{l+1}")
                        pack_y = None
                    else:
                        pack = None
                        pack_y = pk.tile([P, G, OUTD], F32, tag="packy")
                    for b in range(g0, g1):
                        bi = b - g0
                        acc = ps.tile([P, nch_w], F32, tag="agg")
                        chunks = [bi * te + j for j in range(te)] + [nche + bi * to + j for j in range(to)]
                        for ci, ch in enumerate(chunks):
                            nc.tensor.matmul(acc[:], m[:, ch, :], g[:, ch, 0:nch_w],
                                             start=(ci == 0), stop=(ci == len(chunks) - 1))
                        finalize_block(l, acc, b, bi, pack, pack_y)
                    if l < 2:
                        nl = l + 1
                        nhn = NH[nl]
                        on = pack[:, 0:gn, 0:nhn * (HID + 1)].rearrange("p g (h c) -> p g h c", h=nhn)[:, :, :, HID:HID + 1]
                        nc.vector.memset(on, 1.0)
                        stage_group(nl, g0, gn, pack)
                    else:
                        nc.sync.dma_start(
                            y_out[g0 * P:g1 * P, :].rearrange("(g q) w -> q g w", g=gn),
                            pack_y[:, 0:gn, :])

    nc.compile()
    return nc


# ----------------------------------------------------------------------------
# Host wrapper
# ----------------------------------------------------------------------------

_CACHE = {}


def _scatter_A(a_s, a_d):
    nh, hc = a_s.shape
    A = np.zeros((nh * hc, 2 * nh), np.float32)
    for hd in range(nh):
        A[hd * hc:(hd + 1) * hc, hd] = a_s[hd]
        A[hd * hc:(hd + 1) * hc, nh + hd] = a_d[hd]
    return A


def prepare(inputs, nblk, te, to, G, n_nodes, h1b):
    """Build (nc, in_maps, new_id) without executing."""
    import os
    T = te + to
    nloc = nblk * P
    npair_c = nloc // 2
    npair = NCORES * npair_c
    h1 = h1b * 64
    h2 = npair_c - h1
    groups = [(i, min(i + G, nblk)) for i in range(0, nblk, G)]
    edge_index = np.asarray(inputs["edge_index"])
    x = np.asarray(inputs["x"], np.float32)

    new_id, plans = preprocess(edge_index, n_nodes, nblk, te, to)

    skip_ag = bool(int(os.environ.get("K_SKIP_AG", "0")))
    skip_gather = bool(int(os.environ.get("K_SKIP_GATHER", "0")))
    noop = bool(int(os.environ.get("K_NOOP", "0")))
    repeat = int(os.environ.get("K_REPEAT", "1"))
    h1b = int(os.environ.get("K_H1B", str(h1b)))
    h1 = h1b * 64
    h2 = npair_c - h1
    pool_scale = bool(int(os.environ.get("K_POOL_SCALE", "0")))
    G = int(os.environ.get("K_G", str(G)))
    no_scale = bool(int(os.environ.get("K_NOSCALE", "0")))
    no_m = bool(int(os.environ.get("K_NOM", "0")))
    NO_EE[0] = bool(int(os.environ.get("K_NOEE", "0")))
    key = (nblk, te, to, G, h1b, skip_ag, skip_gather, noop, repeat, pool_scale,
           no_scale, no_m, NO_EE[0])
    if key not in _CACHE:
        _CACHE[key] = build_kernel(dict(nblk=nblk, te=te, to=to, g=G, h1b=h1b,
                                        n_nodes=n_nodes, skip_ag=skip_ag,
                                        skip_gather=skip_gather, noop=noop,
                                        repeat=repeat, pool_scale=pool_scale,
                                        no_scale=no_scale, no_m=no_m))
    nc = _CACHE[key]

    # table pair-row id for a global new node id (vectorized)
    def pair_row_v(gids):
        c = gids // nloc
        loc = gids % nloc
        lp = (loc // P) * 64 + (loc % P) // 2
        if skip_ag:
            return lp  # local copy keeps core order
        return np.where(lp < h1, c * h1 + lp, NCORES * h1 + c * h2 + (lp - h1))

    iota = np.tile(np.arange(P, dtype=BF), (P, 1))
    ident = np.eye(P, dtype=np.float32)
    Wl = {f"W{l}": np.asarray(inputs[f"W{l}"], np.float32) for l in range(3)}
    Al = {f"A{l}": _scatter_A(np.asarray(inputs[f"as{l}"], np.float32),
                              np.asarray(inputs[f"ad{l}"], np.float32)) for l in range(3)}
    bnl = {f"bn{l}": np.stack([np.asarray(inputs[k + str(l)], np.float32)
                               for k in ("b", "g", "bt", "m", "v")]) for l in range(3)}

    in_maps = []
    for c in range(NCORES):
        pl = plans[c]
        xs = np.zeros((nloc, F_IN), np.float32)
        ids = new_id - c * nloc
        mine = (ids >= 0) & (ids < nloc)
        xs[ids[mine]] = x[mine]
        # streams in tile order
        fe_l, fo_l, dsl_l, dst_t = [], [], [], []
        for g0, g1 in groups:
            blks = list(range(g0, g1))
            ie = np.concatenate([pl["idx_e"][b] for b in blks])
            io = np.concatenate([pl["idx_o"][b] for b in blks])
            de = np.concatenate([pl["ds_e"][b] for b in blks])
            do = np.concatenate([pl["ds_o"][b] for b in blks])
            fe_l.append(np.where(ie >= 0, pair_row_v(np.maximum(ie, 0)), npair))
            fo_l.append(np.where(io >= 0, pair_row_v(np.maximum(io, 0)), npair))
            dsl_l.append(np.concatenate([de, do]))
        idx_e = _wrap_idx(np.concatenate(fe_l))
        idx_o = _wrap_idx(np.concatenate(fo_l))
        dsall = np.concatenate(dsl_l)             # [nblk*T*P] dst slots, tile order
        dslot = np.ascontiguousarray(dsall.reshape(-1, P).T.astype(BF))
        mT = np.zeros((P, dsall.size), BF)
        mT[dsall, np.arange(dsall.size)] = 1.0
        im = dict(x_slice=xs, idx_e=idx_e, idx_o=idx_o,
                  dslot=dslot, mT=mT,
                  Wc=np.asarray(inputs["Wc"], np.float32),
                  bc=np.asarray(inputs["bc"], np.float32).reshape(1, OUTD),
                  ident=ident, iota=iota)
        im.update(Wl)
        im.update(Al)
        im.update(bnl)
        in_maps.append(im)

    return nc, in_maps, new_id


def run(inputs, nblk, te, to, G, n_nodes, h1b):
    nc, in_maps, new_id = prepare(inputs, nblk, te, to, G, n_nodes, h1b)
    res = run_bass_kernel_spmd(nc, in_maps, core_ids=list(range(NCORES)), trace=False)
    y_cat = np.concatenate([r["y"] for r in res.results])  # [NCORES*nloc, 2]
    return y_cat[new_id], res


CFG = dict(nblk=49, te=9, to=9, G=2, n_nodes=N_FULL, h1b=24)


def kernel(**inputs) -> np.ndarray:
    out, _ = run(inputs, **CFG)
    return out.astype(np.float32)


# revision 32
# speedup vs baseline: 1.2001x; 1.2001x over previous
"""GAT (3-layer) Trainium2 Bass kernel, 8-way node-sharded. v2.

Self-contained: host preprocessing (graph partitioning, relabeling, edge
stream construction) + Bass/Tile kernel + gather/unshard.

Strategy (v2):
  - Relabel nodes so core c owns new ids [c*NLOC, (c+1)*NLOC); blocks of 128
    dst nodes; per-block uniform chunk quotas (TE even-src + TO odd-src
    chunks of 128 edges each) with sentinel padding.
  - Single pair-packed gather table per layer (row = even|odd node payload,
    stride 640 bf16 cols L0/L1, 256 cols L2). One AllGather per layer, split
    in two halves so the first half overlaps the tail of the previous layer's
    edge phase. Table rows hold [4x(64 feats + ones) | s_src f32] per node.
  - Per-edge s_dst is computed ON-CHIP: a transposed one-hot (mT, built from
    a partition-broadcast dslotT stream + per-partition iota is_equal) times
    the block's s_dst vector (kept in a persistent SBUF tile) on the PE.
    This removes the per-edge s_dst DMA gather (desc-rate-bound) entirely.
  - Edge phase: dma_gather of per-edge rows (parity = column offset in the
    pair row), exp(lrelu(s_src+s_dst)) per edge, features scaled by per-head
    ee, one-hot matmul on PE accumulates per-dst sums + softmax denominators
    in PSUM.
  - Finalize per block: normalize, bias+BN affine, ELU, dense matmul for the
    next layer via PE transposes, next-layer attention scalars via W@A.
"""
import numpy as np
import ml_dtypes
from contextlib import ExitStack

NO_EE = [False]
NO_FIN = [False]
NO_AGG = [False]

import concourse.bacc as bacc
import concourse.bass as bass
import concourse.mybir as mybir
import concourse.tile as tile
from concourse.bass_utils import run_bass_kernel_spmd

P = 128
NCORES = 8
EPS_BN = 1e-5
NEG = -1e38
F32 = mybir.dt.float32
BF16 = mybir.dt.bfloat16
I16 = mybir.dt.int16
BF = ml_dtypes.bfloat16

# Full-problem constants (matches reference.py / spec.json)
N_FULL, E_FULL, F_IN, HID, HEADS, OUTD = 50000, 800000, 128, 64, 4, 2


# ----------------------------------------------------------------------------
# Host preprocessing
# ----------------------------------------------------------------------------

def preprocess(edge_index, n_nodes, nblk, te, to):
    """Graph partitioning + edge stream construction.

    Returns (new_id [n_nodes], plans per core). All integer index work only.
    """
    T = te + to
    nloc = nblk * P
    cap_e, cap_o = te * P, to * P
    src = np.concatenate([edge_index[0].astype(np.int64), np.arange(n_nodes)])
    dst = np.concatenate([edge_index[1].astype(np.int64), np.arange(n_nodes)])
    deg = np.bincount(dst, minlength=n_nodes)

    # nodes -> cores, balancing in-edge load
    order = np.argsort(-deg, kind="stable")
    cap_nodes = n_nodes // NCORES
    core_of = np.empty(n_nodes, np.int64)
    loads = np.zeros(NCORES, np.int64)
    counts = np.zeros(NCORES, np.int64)
    for n in order:
        avail = np.flatnonzero(counts < cap_nodes)
        c = avail[np.argmin(loads[avail])]
        core_of[n] = c
        loads[c] += deg[n]
        counts[c] += 1

    # per core: nodes -> blocks, balancing block load
    new_id = np.full(n_nodes, -1, np.int64)
    for c in range(NCORES):
        nodes = np.flatnonzero(core_of == c)
        nodes = nodes[np.argsort(-deg[nodes], kind="stable")]
        bload = np.zeros(nblk, np.int64)
        bcount = np.zeros(nblk, np.int64)
        slot_ctr = np.zeros(nblk, np.int64)
        for n in nodes:
            avail = np.flatnonzero(bcount < P)
            b = avail[np.argmin(bload[avail])]
            bload[b] += deg[n]
            bcount[b] += 1
            new_id[n] = c * nloc + b * P + slot_ctr[b]
            slot_ctr[b] += 1
        if bload.max() > T * P:
            raise RuntimeError(f"block overflow: {bload.max()} > {T*P}")

    nsrc = new_id[src]
    ndst = new_id[dst]
    ngb = NCORES * nblk

    # parity repair: per global block, even-src count <= cap_e, odd <= cap_o
    rng = np.random.default_rng(1)
    for _ in range(20000):
        gblk = ndst // P
        ec = np.bincount(gblk[(nsrc & 1) == 0], minlength=ngb)
        tc = np.bincount(gblk, minlength=ngb)
        bad = np.flatnonzero((ec > cap_e) | ((tc - ec) > cap_o))
        if not len(bad):
            break
        b = bad[0]
        par = 0 if ec[b] > cap_e else 1
        eidx = np.flatnonzero(gblk == b)
        cand = eidx[(nsrc[eidx] & 1) == par]
        sn = nsrc[cand[rng.integers(len(cand))]]
        sb = (sn // P) * P
        partners = np.arange(sb + (1 - par), sb + P, 2)
        pn = partners[rng.integers(len(partners))]
        for arr in (nsrc, ndst):
            ms, mp = arr == sn, arr == pn
            arr[ms] = pn
            arr[mp] = sn
        os_ = np.flatnonzero(new_id == sn)[0]
        op_ = np.flatnonzero(new_id == pn)[0]
        new_id[os_], new_id[op_] = pn, sn
    else:
        raise RuntimeError("parity repair failed")

    # per-core streams in tile order:
    # group-major; within a group of G blocks: [all even chunks | all odd chunks]
    plans = []
    for c in range(NCORES):
        sel = (ndst >= c * nloc) & (ndst < (c + 1) * nloc)
        es, ed = nsrc[sel], ndst[sel] - c * nloc
        b = ed // P
        par = (es & 1).astype(np.int64)
        o = np.lexsort((es, par, b))
        es, ed, b, par = es[o], ed[o], b[o], par[o]
        # per-block, per-parity slot ranges (block-major natural order first)
        idx_e = np.full((nblk, cap_e), -1, np.int64)
        ds_e = np.zeros((nblk, cap_e), np.int64)
        idx_o = np.full((nblk, cap_o), -1, np.int64)
        ds_o = np.zeros((nblk, cap_o), np.int64)
        for blk in range(nblk):
            m = b == blk
            for pp, idx_a, ds_a, cap in ((0, idx_e, ds_e, cap_e), (1, idx_o, ds_o, cap_o)):
                mm = m & (par == pp)
                k = int(mm.sum())
                assert k <= cap
                idx_a[blk, :k] = es[mm]
                ds_a[blk, :k] = ed[mm] % P
        plans.append(dict(idx_e=idx_e, ds_e=ds_e, idx_o=idx_o, ds_o=ds_o))
    return new_id, plans


def _wrap_idx(a):
    """[k] int -> wrapped [128, k/16] int16 (16-partition wrap, replicated x8)."""
    w = a.reshape(-1, 16).T.astype(np.int16)
    return np.ascontiguousarray(np.tile(w, (8, 1)))


# ----------------------------------------------------------------------------
# Bass kernel builder (v2)
# ----------------------------------------------------------------------------

def build_kernel(cfg):
    """cfg: dict(nblk, te, to, g, n_nodes[, h1b, repeat, noop, skip_gather,
    skip_ag]). Returns nc."""
    nblk, te, to, G = cfg["nblk"], cfg["te"], cfg["to"], cfg["g"]
    h1b = cfg["h1b"]                 # blocks in AG half 1 (split at group bdry)
    skip_ag = cfg.get("skip_ag", False)
    skip_gather = cfg.get("skip_gather", False)
    repeat = cfg.get("repeat", 1)
    noop = cfg.get("noop", False)
    pool_scale = cfg.get("pool_scale", False)
    no_scale = cfg.get("no_scale", False)
    no_m = cfg.get("no_m", False)
    T = te + to
    nloc = nblk * P
    npair_c = nloc // 2              # pairs per core (incl dummies)
    npair = NCORES * npair_c         # pair rows in the table (excl sentinel)
    h1 = h1b * (P // 2)              # pair rows per core in AG half 1
    h2 = npair_c - h1
    groups = [(i, min(i + G, nblk)) for i in range(0, nblk, G)]

    NODEW = [268, 268, 68]           # bf16 cols of one node's payload
    STRIDE = [640, 640, 256]         # pair-row stride (bf16 cols)
    ELEM = [384, 384, 128]           # gather elem (bf16 cols)
    SOFF = [260, 260, 66]            # s_src f32 at bf16 col (4B aligned)
    NCH = [HEADS * (HID + 1), HEADS * (HID + 1), HID + 1]  # agg matmul width
    NH = [HEADS, HEADS, 1]

    nc = bacc.Bacc("TRN2", num_devices=NCORES, num_swdge_queues=4)
    dt = nc.dram_tensor

    # ---- inputs
    x_in = dt("x_slice", [nloc, F_IN], F32, kind="ExternalInput")
    W_in = [dt(f"W{l}", [F_IN if l == 0 else HEADS * HID, (HEADS if l < 2 else 1) * HID], F32, kind="ExternalInput") for l in range(3)]
    Wc_in = dt("Wc", [HID, OUTD], F32, kind="ExternalInput")
    A_in = [dt(f"A{l}", [(HEADS if l < 2 else 1) * HID, 2 * (HEADS if l < 2 else 1)], F32, kind="ExternalInput") for l in range(3)]
    bn_in = [dt(f"bn{l}", [5, (HEADS if l < 2 else 1) * HID], F32, kind="ExternalInput") for l in range(3)]  # rows: b,g,bt,m,v
    bc_in = dt("bc", [1, OUTD], F32, kind="ExternalInput")
    ident_in = dt("ident", [P, P], F32, kind="ExternalInput")
    iota_in = dt("iota", [P, P], BF16, kind="ExternalInput")

    n_e = te * P
    n_o = to * P
    idx_e_in = dt("idx_e", [P, nblk * n_e // 16], I16, kind="ExternalInput")
    idx_o_in = dt("idx_o", [P, nblk * n_o // 16], I16, kind="ExternalInput")
    dslot_in = dt("dslot", [P, nblk * T], BF16, kind="ExternalInput")
    mT_in = dt("mT", [P, nblk * T * P], BF16, kind="ExternalInput")

    y_out = dt("y", [nloc, OUTD], F32, kind="ExternalOutput")

    # ---- internal DRAM: flat pair tables + staging
    tabf = [dt(f"tab{l}", [1, (npair + 3) * STRIDE[l]], BF16, kind="Internal",
               addr_space="Shared") for l in range(3)]
    stg = [dt(f"stg{l}", [npair_c, STRIDE[l]], BF16, kind="Internal") for l in range(3)]

    rg = [list(range(NCORES))]

    if noop:
        with tile.TileContext(nc) as tc, ExitStack() as ctx:
            p0 = ctx.enter_context(tc.tile_pool(name="noop", bufs=1))
            t0 = p0.tile([P, OUTD], F32)
            nc.vector.memset(t0[:], 0.0)
            nc.sync.dma_start(y_out[0:P, :], t0[:])
        nc.compile()
        return nc

    def tab_rows(l, coloff, elemw):
        """View of tabf[l] as pair rows [npair+1, elemw] starting at coloff."""
        s = STRIDE[l]
        return tabf[l][0:1, coloff:coloff + (npair + 1) * s].rearrange(
            "o (n w) -> (o n) w", w=s)[:, 0:elemw]

    with tile.TileContext(nc) as tc, ExitStack() as ctx:
        cst = ctx.enter_context(tc.tile_pool(name="cst", bufs=1))
        wrk = ctx.enter_context(tc.tile_pool(name="wrk", bufs=2))
        pk = ctx.enter_context(tc.tile_pool(name="pk", bufs=2))
        ps = ctx.enter_context(tc.tile_pool(name="ps", bufs=2, space="PSUM"))
        ps1 = ctx.enter_context(tc.tile_pool(name="ps1", bufs=1, space="PSUM"))
        psd = ctx.enter_context(tc.tile_pool(name="psd", bufs=2, space="PSUM"))
        setup_ctx = ExitStack()
        stp = setup_ctx.enter_context(tc.tile_pool(name="stp", bufs=1))

        # ---- constants to SBUF
        ident = cst.tile([P, P], F32)
        nc.sync.dma_start(ident[:], ident_in[:])
        ones1 = cst.tile([1, P], BF16)
        nc.vector.memset(ones1[:], 1.0)

        def bcast_row(dst_sb, row_ap, w):
            # dst_sb[p, :w] = row_ap[0, :w] for all partitions, via K=1 matmul
            bp = ps1.tile([P, 512], F32, tag="tph")
            rb = stp.tile([1, 512], BF16, tag="rbf")
            nc.vector.tensor_copy(rb[:, :w], row_ap)
            nc.tensor.matmul(bp[:, :w], ones1[:], rb[:, :w], start=True, stop=True)
            nc.vector.tensor_copy(dst_sb, bp[:, :w])
        iota = cst.tile([P, P], BF16)
        nc.sync.dma_start(iota[:], iota_in[:])
        dslot_sb = cst.tile([P, nblk * T], BF16)
        nc.sync.dma_start(dslot_sb[:], dslot_in[:])
        bc_rep = cst.tile([P, OUTD], F32)
        bcl = stp.tile([1, OUTD], F32, tag="bcl")
        nc.sync.dma_start(bcl[:], bc_in[:])
        bcast_row(bc_rep[:], bcl[0:1, :], OUTD)

        # persistent per-block s_dst (bf16), overwritten each layer
        sdst_all = cst.tile([P, nblk, HEADS], BF16)

        # per-layer weight tiles (bf16) + WA + BN affine
        Wt = []     # list of [K-chunk bf16 tiles]  (rhs for dense)
        WAt = []    # [K-chunk [128, 2*nh] bf16]
        sc_rep = []
        sh_rep = []
        for l in range(3):
            fin = F_IN if l == 0 else HEADS * HID
            fout = (HEADS if l < 2 else 1) * HID
            nkin = fin // P
            # load W f32, cast to bf16 chunk tiles
            wf = stp.tile([P, (fin // P) * fout], F32, tag=f"wload{l}")
            nc.sync.dma_start(wf[:].rearrange("p (k f) -> p k f", k=nkin),
                              W_in[l].rearrange("(k p) f -> p k f", p=P))
            wb = cst.tile([P, (fin // P) * fout], BF16, tag=f"wb{l}")
            nc.vector.tensor_copy(wb[:], wf[:])
            Wt.append([wb[:, k * fout:(k + 1) * fout] for k in range(nkin)])

            # WT chunks (f32) for WA = W.T-free computation: WA[fin,2nh]
            nchT = (fout + P - 1) // P
            wT = stp.tile([P, nchT * fin], F32, tag=f"wT{l}")
            for ki in range(nkin):          # W row chunk (fin dim)
                for kj in range(nchT):      # W col chunk (fout dim)
                    cw = min(P, fout - kj * P)
                    tp = ps1.tile([P, 512], F32, tag="tph")
                    nc.tensor.transpose(tp[:cw, :P], wf[:, ki * fout + kj * P: ki * fout + kj * P + cw], ident[:])
                    nc.vector.tensor_copy(wT[:cw, kj * fin + ki * P: kj * fin + (ki + 1) * P], tp[:cw, :P])
            nh = NH[l]
            fo_p = min(P, fout)
            af = stp.tile([P, nchT * 2 * nh], F32, tag=f"aload{l}")
            if fout >= P:
                nc.sync.dma_start(af[:].rearrange("p (k f) -> p k f", k=nchT),
                                  A_in[l].rearrange("(k p) f -> p k f", p=P))
            else:
                nc.sync.dma_start(af[:fo_p, 0:2 * nh], A_in[l][:])
            # WA [fin, 2nh] = sum_kj WT_chunk.T @ A_chunk   (bf16 result)
            wab = cst.tile([P, nkin * 2 * nh], BF16, tag=f"wab{l}")
            for ki in range(nkin):
                wa_ps = ps1.tile([P, 2 * nh], F32, tag="sps")
                for kj in range(nchT):
                    cw = min(P, fout - kj * P)
                    nc.tensor.matmul(wa_ps[:],
                                     wT[:cw, kj * fin + ki * P: kj * fin + (ki + 1) * P],
                                     af[:cw, kj * 2 * nh:(kj + 1) * 2 * nh],
                                     start=(kj == 0), stop=(kj == nchT - 1))
                nc.vector.tensor_copy(wab[:, ki * 2 * nh:(ki + 1) * 2 * nh], wa_ps[:])
            WAt.append([wab[:, k * 2 * nh:(k + 1) * 2 * nh] for k in range(nkin)])

            # BN affine: scale2 = g/sqrt(v+eps); shift2 = (b - m)*scale2 + bt
            bn = stp.tile([1, 5 * fout], F32, tag=f"bnload{l}")
            nc.sync.dma_start(bn[:].rearrange("p (r f) -> p r f", r=5), bn_in[l][None, :, :])
            bnr = [bn[:, i * fout:(i + 1) * fout] for i in range(5)]  # b,g,bt,m,v
            sc1 = stp.tile([1, fout], F32, tag=f"sc1{l}")
            nc.vector.tensor_scalar(out=sc1[:], in0=bnr[4], scalar1=EPS_BN, scalar2=None, op0=mybir.AluOpType.add)
            nc.scalar.activation(sc1[:], sc1[:], mybir.ActivationFunctionType.Sqrt)
            rc = stp.tile([1, fout], F32, tag=f"rc1{l}")
            nc.vector.reciprocal(rc[:], sc1[:])
            nc.vector.tensor_tensor(out=rc[:], in0=rc[:], in1=bnr[1], op=mybir.AluOpType.mult)
            sh1 = stp.tile([1, fout], F32, tag=f"sh1{l}")
            nc.vector.tensor_tensor(out=sh1[:], in0=bnr[0], in1=bnr[3], op=mybir.AluOpType.subtract)
            nc.vector.tensor_tensor(out=sh1[:], in0=sh1[:], in1=rc[:], op=mybir.AluOpType.mult)
            nc.vector.tensor_tensor(out=sh1[:], in0=sh1[:], in1=bnr[2], op=mybir.AluOpType.add)
            screp = cst.tile([P, fout], F32, tag=f"screp{l}")
            bcast_row(screp[:], rc[0:1, :], fout)
            shrep = cst.tile([P, fout], F32, tag=f"shrep{l}")
            bcast_row(shrep[:], sh1[0:1, :], fout)
            sc_rep.append(screp)
            sh_rep.append(shrep)

        wcf = stp.tile([HID, OUTD], F32, tag="wcl")
        nc.sync.dma_start(wcf[:], Wc_in[:])
        Wct = cst.tile([HID, OUTD], BF16)
        nc.vector.tensor_copy(Wct[:], wcf[:])

        # ---- sentinel rows: zero payload, s_src slots NEG (both parities)
        for l in range(3):
            s = STRIDE[l]
            sent = stp.tile([1, s], BF16, tag=f"sent{l}")
            nc.vector.memset(sent[:], 0)
            s32 = sent[:].bitcast(F32)
            nhl = NH[l]
            nc.vector.memset(s32[:, SOFF[l] // 2: SOFF[l] // 2 + nhl], NEG)
            nc.vector.memset(s32[:, (NODEW[l] + SOFF[l]) // 2: (NODEW[l] + SOFF[l]) // 2 + nhl], NEG)
            nc.sync.dma_start(
                tabf[l][0:1, npair * s:(npair + 1) * s], sent[:])
            # zero the spill pad row too (keeps over-reads NaN-free)
            nc.vector.memset(sent[:], 0)
            nc.sync.dma_start(tabf[l][0:1, (npair + 1) * s:(npair + 2) * s], sent[:])

        # ================= per-layer finalize helper ====================
        def finalize_block(l, zsrc_ps, b, b_in_grp, pack, pack_y):
            """zsrc_ps: agg psum [P, NCH[l]] (cols: per-head 64 feats + denom).
            Produces next-layer packed node rows into pack (this group's
            staging tile), or y into pack_y for l==2. Writes sdst_all[:, b]."""
            nh = NH[l]
            fout = nh * HID
            # denominators -> reciprocal
            dn = wrk.tile([P, nh], F32, tag=f"dn{nh}")
            dcols = zsrc_ps[:].rearrange("p (h c) -> p h c", c=HID + 1)[:, :, HID]
            nc.vector.tensor_scalar(out=dn[:], in0=dcols, scalar1=1e-30, scalar2=None, op0=mybir.AluOpType.add)
            rd = wrk.tile([P, nh], F32, tag=f"rd{nh}")
            nc.vector.reciprocal(rd[:], dn[:])
            # normalize * BN-scale fused, then + shift
            h = wrk.tile([P, fout], F32, tag=f"h{fout}")
            for hd in range(nh):
                nc.vector.scalar_tensor_tensor(
                    out=h[:, hd * HID:(hd + 1) * HID],
                    in0=zsrc_ps[:, hd * (HID + 1):hd * (HID + 1) + HID],
                    scalar=rd[:, hd, None],
                    in1=sc_rep[l][:, hd * HID:(hd + 1) * HID],
                    op0=mybir.AluOpType.mult, op1=mybir.AluOpType.mult)
            nc.vector.tensor_tensor(out=h[:], in0=h[:], in1=sh_rep[l][:], op=mybir.AluOpType.add)
            if l < 2:
                u = wrk.tile([P, fout], F32, tag=f"elu_u{fout}")
                nc.vector.tensor_scalar(out=u[:], in0=h[:], scalar1=0.0, scalar2=None, op0=mybir.AluOpType.min)
                nc.scalar.activation(u[:], u[:], mybir.ActivationFunctionType.Exp)
                nc.vector.tensor_scalar(out=h[:], in0=h[:], scalar1=0.0, scalar2=-1.0, op0=mybir.AluOpType.max, op1=mybir.AluOpType.add)
                nc.vector.tensor_tensor(out=h[:], in0=h[:], in1=u[:], op=mybir.AluOpType.add)
            # transpose h -> hT bf16 chunks
            nkin = fout // P if fout >= P else 1
            hT = wrk.tile([P, nkin * P], BF16, tag=f"hT{fout}")
            for k in range(nkin):
                cw = min(P, fout - k * P)
                tp = ps1.tile([P, 512], F32, tag="tph")
                nc.tensor.transpose(tp[:cw, :P], h[:, k * P:k * P + cw], ident[:])
                nc.scalar.copy(hT[:cw, k * P:(k + 1) * P], tp[:cw, 0:P])
            if l == 2:
                # head: y = h2 @ Wc + bc
                yp = ps1.tile([P, OUTD], F32, tag="sps")
                nc.tensor.matmul(yp[:], hT[:HID, 0:P], Wct[:], start=True, stop=True)
                nc.vector.tensor_tensor(out=pack_y[:, b_in_grp, :], in0=yp[:], in1=bc_rep[:], op=mybir.AluOpType.add)
                return
            # dense: hf_next = h @ W_{l+1}; s = h @ WA_{l+1}   (both bf16 hT)
            nl = l + 1
            nhn = NH[nl]
            fon = nhn * HID
            hf = ps.tile([P, fon], F32, tag="hfps")
            sps = ps1.tile([P, 2 * nhn], F32, tag="sps")
            for k in range(nkin):
                nc.tensor.matmul(hf[:], hT[:, k * P:(k + 1) * P], Wt[nl][k], start=(k == 0), stop=(k == nkin - 1))
            for k in range(nkin):
                nc.tensor.matmul(sps[:], hT[:, k * P:(k + 1) * P], WAt[nl][k], start=(k == 0), stop=(k == nkin - 1))
            # pack node rows: [hf_hd | 1.0]*nh + s_src f32
            nc.scalar.copy(
                pack[:, b_in_grp, 0:nhn * (HID + 1)].rearrange("p (h c) -> p h c", h=nhn)[:, :, 0:HID],
                hf[:].rearrange("p (h c) -> p h c", h=nhn))
            pk32 = pack[:].bitcast(F32)
            so = SOFF[nl] // 2
            nc.vector.tensor_copy(pk32[:, b_in_grp, so:so + nhn], sps[:, 0:nhn])
            # next layer's s_dst -> persistent sbuf (bf16)
            nc.vector.tensor_copy(sdst_all[:, b, 0:nhn], sps[:, nhn:2 * nhn])

        def stage_group(l, g0, gn, pack):
            """Write group's pack (node rows) into stg[l] pair rows."""
            nw = NODEW[l]
            for par in (0, 1):
                nc.sync.dma_start(
                    stg[l][g0 * 64:g0 * 64 + gn * 64, par * nw:(par + 1) * nw]
                    .rearrange("(g q) w -> q g w", g=gn),
                    pack[par::2, 0:gn, 0:nw])

        def do_ag(l):
            s = STRIDE[l]
            if skip_ag:
                nc.sync.dma_start(
                    tabf[l][0:1, 0:npair_c * s].rearrange("o (n w) -> (o n) w", w=s),
                    stg[l][:])
                return
            nc.gpsimd.collective_compute(
                "AllGather", mybir.AluOpType.bypass,
                ins=[stg[l][0:h1, :]],
                outs=[tabf[l][0:1, 0:NCORES * h1 * s]], replica_groups=rg)
            nc.gpsimd.collective_compute(
                "AllGather", mybir.AluOpType.bypass,
                ins=[stg[l][h1:npair_c, :]],
                outs=[tabf[l][0:1, NCORES * h1 * s:NCORES * npair_c * s]],
                replica_groups=rg)

        setup_ctx.close()

        for _rep in range(repeat):
            # ================= L0 prep: x -> table0 rows =====================
            for g0, g1 in groups:
                gn = g1 - g0
                pack = pk.tile([P, G, NODEW[0]], BF16, tag="pack0")
                nc.vector.memset(pack[:, :gn, :], 0)
                for b in range(g0, g1):
                    bi = b - g0
                    xt = wrk.tile([P, F_IN], F32, tag="xt")
                    nc.sync.dma_start(xt[:], x_in[b * P:(b + 1) * P, :])
                    xT = wrk.tile([P, F_IN], BF16, tag="xT")
                    tp = ps1.tile([P, 512], F32, tag="tph")
                    nc.tensor.transpose(tp[:, 0:P], xt[:], ident[:])
                    nc.scalar.copy(xT[:], tp[:, 0:P])
                    hf = ps.tile([P, HEADS * HID], F32, tag="hfps")
                    sps = ps1.tile([P, 2 * HEADS], F32, tag="sps")
                    nc.tensor.matmul(hf[:], xT[:], Wt[0][0], start=True, stop=True)
                    nc.tensor.matmul(sps[:], xT[:], WAt[0][0], start=True, stop=True)
                    nc.scalar.copy(
                        pack[:, bi, 0:HEADS * (HID + 1)].rearrange("p (h c) -> p h c", h=HEADS)[:, :, 0:HID],
                        hf[:].rearrange("p (h c) -> p h c", h=HEADS))
                    pk32 = pack[:].bitcast(F32)
                    so = SOFF[0] // 2
                    nc.vector.tensor_copy(pk32[:, bi, so:so + HEADS], sps[:, 0:HEADS])
                    nc.vector.tensor_copy(sdst_all[:, b, :], sps[:, HEADS:2 * HEADS])
                # ones cols
                on = pack[:, 0:gn, 0:HEADS * (HID + 1)].rearrange("p g (h c) -> p g h c", h=HEADS)[:, :, :, HID:HID + 1]
                nc.vector.memset(on, 1.0)
                stage_group(0, g0, gn, pack)

            # ================= layers =====================
            for l in range(3):
                nh = NH[l]
                elem = ELEM[l]
                nch_w = NCH[l]
                soff32 = SOFF[l] // 2
                do_ag(l)

                for gi, (g0, g1) in enumerate(groups):
                    gn = g1 - g0
                    nche = gn * te       # even chunks in group
                    ncho = gn * to
                    ncht = nche + ncho
                    # chunk order in tiles: [even chunks | odd chunks]
                    ixe = wrk.tile([P, nche * 8], I16, tag="ixe")
                    nc.sync.dma_start(ixe[:], idx_e_in[:, g0 * n_e // 16:(g0 * n_e + nche * P) // 16])
                    ixo = wrk.tile([P, ncho * 8], I16, tag="ixo")
                    nc.sync.dma_start(ixo[:], idx_o_in[:, g0 * n_o // 16:(g0 * n_o + ncho * P) // 16])

                    # mT (one-hot, dst on partitions) streamed from host
                    mt = wrk.tile([P, ncht, P], BF16, tag="mt")
                    nc.sync.dma_start(mt[:].rearrange("p c q -> p (c q)"),
                                      mT_in[:, g0 * T * P:(g0 * T + ncht) * P])

                    g = wrk.tile([P, G * T, elem], BF16, tag="g")
                    if skip_gather:
                        nc.vector.memset(g[:, 0:ncht, :], 0)
                    else:
                        nc.gpsimd.dma_gather(g[:, 0:nche, :], tab_rows(l, 0, elem),
                                             ixe[:], nche * P, nche * P, elem,
                                             elem_step=STRIDE[l], single_packet=False,
                                             queue_num=(2 * gi) % 4)
                        nc.gpsimd.dma_gather(g[:, nche:ncht, :], tab_rows(l, NODEW[l], elem),
                                             ixo[:], ncho * P, ncho * P, elem,
                                             elem_step=STRIDE[l], single_packet=False,
                                             queue_num=(2 * gi + 1) % 4)

                    # per-edge s_dst via mT @ sdst_blk on PE
                    no_ee = NO_EE[0]
                    sdp = psd.tile([P, G * T * HEADS], F32, tag="sdp")
                    def blk_of(ci):
                        return g0 + (ci // te if ci < nche else (ci - nche) // to)
                    for ci in range(0 if not no_ee else ncht, ncht):
                        nc.tensor.matmul(sdp[:, ci * nh:(ci + 1) * nh],
                                         mt[:, ci, :], sdst_all[:, blk_of(ci), 0:nh],
                                         start=True, stop=True)
                    sde = wrk.tile([P, G * T, HEADS], F32, tag="sde")
                    if not no_ee:
                        nc.scalar.copy(sde[:, 0:ncht, 0:nh].rearrange("p c h -> p (c h)"),
                                   sdp[:, 0:ncht * nh])

                    # e = lrelu(s_src + s_dst); ee = exp(e)  (bf16)
                    g32 = g[:].bitcast(F32)
                    e_t = wrk.tile([P, G * T, HEADS], F32, tag="e_t")
                    if not no_ee:
                        nc.vector.tensor_tensor(out=e_t[:, 0:ncht, 0:nh], in0=sde[:, 0:ncht, 0:nh],
                                                in1=g32[:, 0:ncht, soff32:soff32 + nh], op=mybir.AluOpType.add)
                        nc.vector.scalar_tensor_tensor(out=e_t[:, 0:ncht, 0:nh], in0=e_t[:, 0:ncht, 0:nh],
                                                       scalar=0.2, in1=e_t[:, 0:ncht, 0:nh],
                                                       op0=mybir.AluOpType.mult, op1=mybir.AluOpType.max)
                    ee = wrk.tile([P, G * T, HEADS], BF16, tag="ee")
                    if no_ee:
                        nc.vector.memset(ee[:, 0:ncht, 0:nh], 1.0)
                    else:
                        nc.scalar.activation(ee[:, 0:ncht, 0:nh], e_t[:, 0:ncht, 0:nh], mybir.ActivationFunctionType.Exp)

                    # scale gathered features by per-head ee (in place),
                    # split across DVE and Pool
                    for hd in range(0 if not no_scale else nh, nh):
                        eng = nc.gpsimd if (pool_scale and nh > 1 and hd >= 2) else nc.vector
                        eng.tensor_tensor(
                            out=g[:, 0:ncht, hd * (HID + 1):(hd + 1) * (HID + 1)],
                            in0=g[:, 0:ncht, hd * (HID + 1):(hd + 1) * (HID + 1)],
                            in1=ee[:, 0:ncht, hd, None].to_broadcast([P, ncht, HID + 1]),
                            op=mybir.AluOpType.mult)

                    # one-hot m (edges on partitions) for aggregation
                    m = wrk.tile([P, G * T, P], BF16, tag="m")
                    if no_m:
                        nc.vector.memset(m[:, 0:ncht, :], 0)
                    else:
                        nc.vector.tensor_tensor(
                            out=m[:, 0:ncht, :],
                            in0=iota[:, None, :].to_broadcast([P, ncht, P]),
                            in1=dslot_sb[:, (g0 * T):(g0 * T) + ncht, None].to_broadcast([P, ncht, P]),
                            op=mybir.AluOpType.is_equal)

                    # aggregate per block
                    if l < 2:
                        pack = pk.tile([P, G, NODEW[l + 1]], BF16, tag=f"packL{l+1}")
                        pack_y = None
                    else:
                        pack = None
                        pack_y = pk.tile([P, G, OUTD], F32, tag="packy")
                    for b in range(g0, g1):
                        bi = b - g0
                        acc = ps.tile([P, nch_w], F32, tag="agg")
                        chunks = [bi * te + j for j in range(te)] + [nche + bi * to + j for j in range(to)]
                        if NO_AGG[0]:
                            nc.tensor.matmul(acc[:], m[:, chunks[0], :], g[:, chunks[0], 0:nch_w],
                                             start=True, stop=True)
                        else:
                            for ci, ch in enumerate(chunks):
                                nc.tensor.matmul(acc[:], m[:, ch, :], g[:, ch, 0:nch_w],
                                                 start=(ci == 0), stop=(ci == len(chunks) - 1))
                        if NO_FIN[0]:
                            if l == 2:
                                nc.vector.memset(pack_y[:, bi, :], 0.0)
                            else:
                                nc.vector.memset(pack[:, bi, :], 0)
                        else:
                            finalize_block(l, acc, b, bi, pack, pack_y)
                    if l < 2:
                        nl = l + 1
                        nhn = NH[nl]
                        on = pack[:, 0:gn, 0:nhn * (HID + 1)].rearrange("p g (h c) -> p g h c", h=nhn)[:, :, :, HID:HID + 1]
                        nc.vector.memset(on, 1.0)
                        stage_group(nl, g0, gn, pack)
                    else:
                        nc.sync.dma_start(
                            y_out[g0 * P:g1 * P, :].rearrange("(g q) w -> q g w", g=gn),
                            pack_y[:, 0:gn, :])

    nc.compile()
    return nc


# ----------------------------------------------------------------------------
# Host wrapper
# ----------------------------------------------------------------------------

_CACHE = {}


def _scatter_A(a_s, a_d):
    nh, hc = a_s.shape
    A = np.zeros((nh * hc, 2 * nh), np.float32)
    for hd in range(nh):
        A[hd * hc:(hd + 1) * hc, hd] = a_s[hd]
        A[hd * hc:(hd + 1) * hc, nh + hd] = a_d[hd]
    return A


def prepare(inputs, nblk, te, to, G, n_nodes, h1b):
    """Build (nc, in_maps, new_id) without executing."""
    import os
    T = te + to
    nloc = nblk * P
    npair_c = nloc // 2
    npair = NCORES * npair_c
    h1 = h1b * 64
    h2 = npair_c - h1
    groups = [(i, min(i + G, nblk)) for i in range(0, nblk, G)]
    edge_index = np.asarray(inputs["edge_index"])
    x = np.asarray(inputs["x"], np.float32)

    new_id, plans = preprocess(edge_index, n_nodes, nblk, te, to)

    skip_ag = bool(int(os.environ.get("K_SKIP_AG", "0")))
    skip_gather = bool(int(os.environ.get("K_SKIP_GATHER", "0")))
    noop = bool(int(os.environ.get("K_NOOP", "0")))
    repeat = int(os.environ.get("K_REPEAT", "1"))
    h1b = int(os.environ.get("K_H1B", str(h1b)))
    h1 = h1b * 64
    h2 = npair_c - h1
    pool_scale = bool(int(os.environ.get("K_POOL_SCALE", "0")))
    G = int(os.environ.get("K_G", str(G)))
    no_scale = bool(int(os.environ.get("K_NOSCALE", "0")))
    no_m = bool(int(os.environ.get("K_NOM", "0")))
    NO_EE[0] = bool(int(os.environ.get("K_NOEE", "0")))
    NO_FIN[0] = bool(int(os.environ.get("K_NOFIN", "0")))
    NO_AGG[0] = bool(int(os.environ.get("K_NOAGG", "0")))
    key = (nblk, te, to, G, h1b, skip_ag, skip_gather, noop, repeat, pool_scale,
           no_scale, no_m, NO_EE[0], NO_FIN[0], NO_AGG[0])
    if key not in _CACHE:
        _CACHE[key] = build_kernel(dict(nblk=nblk, te=te, to=to, g=G, h1b=h1b,
                                        n_nodes=n_nodes, skip_ag=skip_ag,
                                        skip_gather=skip_gather, noop=noop,
                                        repeat=repeat, pool_scale=pool_scale,
                                        no_scale=no_scale, no_m=no_m))
    nc = _CACHE[key]

    # table pair-row id for a global new node id (vectorized)
    def pair_row_v(gids):
        c = gids // nloc
        loc = gids % nloc
        lp = (loc // P) * 64 + (loc % P) // 2
        if skip_ag:
            return lp  # local copy keeps core order
        return np.where(lp < h1, c * h1 + lp, NCORES * h1 + c * h2 + (lp - h1))

    iota = np.tile(np.arange(P, dtype=BF), (P, 1))
    ident = np.eye(P, dtype=np.float32)
    Wl = {f"W{l}": np.asarray(inputs[f"W{l}"], np.float32) for l in range(3)}
    Al = {f"A{l}": _scatter_A(np.asarray(inputs[f"as{l}"], np.float32),
                              np.asarray(inputs[f"ad{l}"], np.float32)) for l in range(3)}
    bnl = {f"bn{l}": np.stack([np.asarray(inputs[k + str(l)], np.float32)
                               for k in ("b", "g", "bt", "m", "v")]) for l in range(3)}

    in_maps = []
    for c in range(NCORES):
        pl = plans[c]
        xs = np.zeros((nloc, F_IN), np.float32)
        ids = new_id - c * nloc
        mine = (ids >= 0) & (ids < nloc)
        xs[ids[mine]] = x[mine]
        # streams in tile order
        fe_l, fo_l, dsl_l, dst_t = [], [], [], []
        for g0, g1 in groups:
            blks = list(range(g0, g1))
            ie = np.concatenate([pl["idx_e"][b] for b in blks])
            io = np.concatenate([pl["idx_o"][b] for b in blks])
            de = np.concatenate([pl["ds_e"][b] for b in blks])
            do = np.concatenate([pl["ds_o"][b] for b in blks])
            fe_l.append(np.where(ie >= 0, pair_row_v(np.maximum(ie, 0)), npair))
            fo_l.append(np.where(io >= 0, pair_row_v(np.maximum(io, 0)), npair))
            dsl_l.append(np.concatenate([de, do]))
        idx_e = _wrap_idx(np.concatenate(fe_l))
        idx_o = _wrap_idx(np.concatenate(fo_l))
        dsall = np.concatenate(dsl_l)             # [nblk*T*P] dst slots, tile order
        dslot = np.ascontiguousarray(dsall.reshape(-1, P).T.astype(BF))
        mT = np.zeros((P, dsall.size), BF)
        mT[dsall, np.arange(dsall.size)] = 1.0
        im = dict(x_slice=xs, idx_e=idx_e, idx_o=idx_o,
                  dslot=dslot, mT=mT,
                  Wc=np.asarray(inputs["Wc"], np.float32),
                  bc=np.asarray(inputs["bc"], np.float32).reshape(1, OUTD),
                  ident=ident, iota=iota)
        im.update(Wl)
        im.update(Al)
        im.update(bnl)
        in_maps.append(im)

    return nc, in_maps, new_id


def run(inputs, nblk, te, to, G, n_nodes, h1b):
    nc, in_maps, new_id = prepare(inputs, nblk, te, to, G, n_nodes, h1b)
    res = run_bass_kernel_spmd(nc, in_maps, core_ids=list(range(NCORES)), trace=False)
    y_cat = np.concatenate([r["y"] for r in res.results])  # [NCORES*nloc, 2]
    return y_cat[new_id], res


CFG = dict(nblk=49, te=9, to=9, G=2, n_nodes=N_FULL, h1b=24)


def kernel(**inputs) -> np.ndarray:
    out, _ = run(inputs, **CFG)
    return out.astype(np.float32)
